# revision 5
# baseline (speedup 1.0000x reference)
"""Trainium2 Bass kernel for nn_LoraLayer (grouped-GEMM LoRA / MoE routing).

Math (see reference):
  xr = x[sorted_ids]                      # tokens regrouped so same-slot rows
                                          # are contiguous (512 rows per slot)
  per module m, slot s:
    t = xr_slot @ (A[m,s] rank-masked)    # [512, 16]
    y = t @ B[m,s]                        # [512, 8192]
  out = concat over modules along columns # [4096, 16384], in xr row order

Sharding: data-parallel over the token dim. With the reference's slot layout
(8 slots x 512 rows) each core owns exactly one slot's row block, so each core
only needs its own slot's (A, B). The row gather + transpose of x is done on
the host as part of the shard step (contraction dim must sit on SBUF
partitions; f32 DMA-transpose doesn't exist on trn2).

Per-core device program:
  xT  [2048, 512]  (hidden on partitions, 16 chunks of 128)
  a   [2, 2048, 16] rank-masked                 -> sbuf [128, 2*16*16]
  b   [2, 16, 8192]                             -> sbuf [16, 8192] x2
  GEMM1: t.T[16, 512] = sum_k a_chunk.T @ x_chunk   (PSUM accum)
  GEMM2: y[128, 512] tiles = (t.T slice).T @ b slice, staged to [128, 8192]
         sbuf tiles, DMA'd out as 4 MB strided writes.
"""

import numpy as np

BS = 4096
HIDDEN = 2048
MAX_RANK = 16
N_SLOTS = 8
N_MOD = 2
OUT = 8192
N_CORES = 8
TPC = BS // N_CORES          # tokens per core = 512
KCH = HIDDEN // 128          # 16 k-chunks
MT = TPC // 128              # 4 token tiles per core
NT = OUT // 512              # 16 output column tiles per module

_CACHE = {}


def _build_bass():
    import concourse.mybir as mybir
    import concourse.tile as tile
    from concourse import bacc

    f32 = mybir.dt.float32
    nc = bacc.Bacc()

    xT = nc.dram_tensor("xT", [HIDDEN, TPC], f32, kind="ExternalInput")
    a = nc.dram_tensor("a", [N_MOD, HIDDEN, MAX_RANK], f32, kind="ExternalInput")
    b = nc.dram_tensor("b", [N_MOD, MAX_RANK, OUT], f32, kind="ExternalInput")
    y = nc.dram_tensor("y", [TPC, N_MOD * OUT], f32, kind="ExternalOutput")

    with tile.TileContext(nc) as tc:
        with (
            tc.tile_pool(name="xp", bufs=1) as xp,
            tc.tile_pool(name="ap", bufs=1) as ap,
            tc.tile_pool(name="bp", bufs=1) as bp,
            tc.tile_pool(name="tp", bufs=2) as tp,
            tc.tile_pool(name="pt", bufs=2, space="PSUM") as pt,
            tc.tile_pool(name="py", bufs=4, space="PSUM") as py,
            tc.tile_pool(name="yp", bufs=2) as yp,
        ):
            # x: [2048, 512] -> sbuf [128, 16, 512]; free block k = k-chunk
            x_sb = xp.tile([128, KCH, TPC], f32)
            nc.sync.dma_start(
                out=x_sb[:], in_=xT[:, :].rearrange("(k p) n -> p k n", p=128)
            )
            # a: [2, 2048, 16] -> sbuf [128, 2, 16, 16]
            a_sb = ap.tile([128, N_MOD, KCH, MAX_RANK], f32)
            nc.sync.dma_start(
                out=a_sb[:], in_=a[:, :, :].rearrange("m (k p) r -> p m k r", p=128)
            )
            # b: one [16, 8192] tile per module (K=rank on partitions)
            b_sb = []
            for m in range(N_MOD):
                bt = bp.tile([MAX_RANK, OUT], f32, tag=f"b{m}")
                nc.sync.dma_start(out=bt[:], in_=b[m])
                b_sb.append(bt)

            for m in range(N_MOD):
                # GEMM1: t.T [16, 512] accumulated over 16 k-chunks
                t_ps = pt.tile([MAX_RANK, TPC], f32)
                for k in range(KCH):
                    nc.tensor.matmul(
                        out=t_ps[:],
                        lhsT=a_sb[:, m, k, :],
                        rhs=x_sb[:, k, :],
                        start=(k == 0),
                        stop=(k == KCH - 1),
                    )
                t_sb = tp.tile([MAX_RANK, TPC], f32)
                nc.vector.tensor_copy(t_sb[:], t_ps[:])

                # GEMM2: per 128-token tile, fill a [128, 8192] stage then DMA
                for mt in range(MT):
                    y_stage = yp.tile([128, OUT], f32, tag="ystage")
                    for nt in range(NT):
                        y_ps = py.tile([128, 512], f32)
                        nc.tensor.matmul(
                            out=y_ps[:],
                            lhsT=t_sb[:, mt * 128 : (mt + 1) * 128],
                            rhs=b_sb[m][:, nt * 512 : (nt + 1) * 512],
                            start=True,
                            stop=True,
                        )
                        dst = y_stage[:, nt * 512 : (nt + 1) * 512]
                        if nt % 3 == 2:
                            nc.scalar.copy(dst, y_ps[:])
                        else:
                            nc.vector.tensor_copy(dst, y_ps[:])
                    nc.sync.dma_start(
                        out=y[mt * 128 : (mt + 1) * 128, m * OUT : (m + 1) * OUT],
                        in_=y_stage[:],
                    )
    nc.compile()
    return nc


def get_bass():
    if "nc" not in _CACHE:
        _CACHE["nc"] = _build_bass()
    return _CACHE["nc"]


def _host_prep(x, A, B, sorted_ids, slot_ranks, slot_offsets):
    """Shard: per-core gathered/transposed x + per-slot rank-masked weights.

    Returns in_maps (list of 8 dicts) or None if the slot layout doesn't
    align with 512-row blocks (then the caller falls back to numpy)."""
    bs = x.shape[0]
    row_slot = (
        np.searchsorted(slot_offsets, np.arange(bs), side="right").astype(np.int64) - 1
    )
    rmask = (
        np.arange(MAX_RANK)[None, :] < slot_ranks[:, None].astype(np.int64)
    ).astype(x.dtype)
    Am = A * rmask[None, :, None, :]  # [m, s, h, r]

    in_maps = []
    for c in range(N_CORES):
        rows = slice(c * TPC, (c + 1) * TPC)
        slots = row_slot[rows]
        s0 = int(slots[0])
        if not (slots == s0).all():
            return None
        ids = sorted_ids[rows]
        in_maps.append(
            {
                "xT": np.ascontiguousarray(x[ids].T),
                "a": np.ascontiguousarray(Am[:, s0]),
                "b": np.ascontiguousarray(B[:, s0]),
            }
        )
    return in_maps


def _numpy_fallback(x, A, B, sorted_ids, slot_ranks, slot_offsets):
    bs = x.shape[0]
    xr = x[sorted_ids]
    row_slot = (
        np.searchsorted(slot_offsets, np.arange(bs), side="right").astype(np.int64) - 1
    )
    rmask = (
        np.arange(MAX_RANK)[None, :] < slot_ranks[:, None].astype(np.int64)
    ).astype(x.dtype)
    Am = A * rmask[None, :, None, :]
    out = np.zeros((bs, N_MOD * OUT), dtype=x.dtype)
    for s in range(N_SLOTS):
        sel = row_slot == s
        if not sel.any():
            continue
        xs = xr[sel]
        for m in range(N_MOD):
            out[sel, m * OUT : (m + 1) * OUT] = (xs @ Am[m, s]) @ B[m, s]
    return out


def kernel(
    x,
    A,
    B,
    sorted_ids,
    slot_counts=None,
    slot_ranks=None,
    slot_offsets=None,
    **_unused,
):
    x = np.asarray(x, dtype=np.float32)
    A = np.asarray(A, dtype=np.float32)
    B = np.asarray(B, dtype=np.float32)
    sorted_ids = np.asarray(sorted_ids)
    slot_ranks = np.asarray(slot_ranks)
    slot_offsets = np.asarray(slot_offsets)

    in_maps = _host_prep(x, A, B, sorted_ids, slot_ranks, slot_offsets)
    if in_maps is None:
        return _numpy_fallback(x, A, B, sorted_ids, slot_ranks, slot_offsets)

    from concourse import bass_utils

    nc = get_bass()
    res = bass_utils.run_bass_kernel_spmd(nc, in_maps, core_ids=list(range(N_CORES)))
    return np.concatenate([r["y"] for r in res.results], axis=0)


if __name__ == "__main__":
    rng = np.random.default_rng(0)
    x = rng.standard_normal((BS, HIDDEN), dtype=np.float32)
    A = rng.standard_normal((N_MOD, N_SLOTS, HIDDEN, MAX_RANK), dtype=np.float32) * 0.02
    B = rng.standard_normal((N_MOD, N_SLOTS, MAX_RANK, OUT), dtype=np.float32) * 0.02
    ids = rng.permutation(BS).astype(np.int32)
    counts = np.full(N_SLOTS, BS // N_SLOTS, np.int32)
    offs = np.concatenate([[0], np.cumsum(counts)[:-1]]).astype(np.int32)
    ranks = np.array([16, 8, 16, 4, 16, 8, 12, 16], np.int32)
    out = kernel(
        x=x, A=A, B=B, sorted_ids=ids, slot_counts=counts,
        slot_ranks=ranks, slot_offsets=offs,
    )
    ref = _numpy_fallback(x, A, B, ids, ranks, offs)
    err = np.abs(out - ref).max() / (np.abs(ref).max() + 1e-30)
    print("max rel err vs numpy:", err)


# revision 25
# speedup vs baseline: 114934.5367x; 114934.5367x over previous
"""Trainium2 Bass kernel for nn_LoraLayer (grouped-GEMM LoRA / MoE routing).

Math (see reference):
  xr = x[sorted_ids]                      # tokens regrouped so same-slot rows
                                          # are contiguous (512 rows per slot)
  per module m, slot s:
    t = xr_slot @ (A[m,s] rank-masked)    # [512, 16]
    y = t @ B[m,s]                        # [512, 8192]
  out = concat over modules along columns # [4096, 16384], in xr row order

Sharding: data-parallel over the token dim. With the reference's slot layout
(8 slots x 512 rows) each core owns exactly one slot's row block, so each core
only needs its own slot's (A, B). The row gather + transpose of x is done on
the host as part of the shard step (contraction dim must sit on SBUF
partitions; f32 DMA-transpose doesn't exist on trn2).

Per-core device program:
  xT  [2048, 512]  (hidden on partitions, 16 chunks of 128)
  a   [2, 2048, 16] rank-masked                 -> sbuf [128, 2*16*16]
  b   [2, 16, 8192]                             -> sbuf [16, 8192] x2
  GEMM1: t.T[16, 512] = sum_k a_chunk.T @ x_chunk   (PSUM accum)
  GEMM2: y[128, 512] tiles = (t.T slice).T @ b slice, staged to [128, 8192]
         sbuf tiles, DMA'd out as 4 MB strided writes.
"""

import numpy as np

BS = 4096
HIDDEN = 2048
MAX_RANK = 16
N_SLOTS = 8
N_MOD = 2
OUT = 8192
N_CORES = 8
TPC = BS // N_CORES          # tokens per core = 512
KCH = HIDDEN // 128          # 16 k-chunks
MT = TPC // 128              # 4 token tiles per core
NT = OUT // 512              # 16 output column tiles per module

_CACHE = {}


def _build_bass(iters=1, mm_dtype="float32", copy_mix=3, packed=False):
    import contextlib

    import concourse.mybir as mybir
    import concourse.tile as tile
    from concourse import bacc

    f32 = mybir.dt.float32
    mdt = getattr(mybir.dt, mm_dtype)
    nc = bacc.Bacc()

    xT = nc.dram_tensor("xT", [HIDDEN, TPC], mdt, kind="ExternalInput")
    a = nc.dram_tensor("a", [N_MOD, HIDDEN, MAX_RANK], mdt, kind="ExternalInput")
    b = nc.dram_tensor("b", [N_MOD, MAX_RANK, OUT], mdt, kind="ExternalInput")
    y = nc.dram_tensor("y", [TPC, N_MOD * OUT], f32, kind="ExternalOutput")

    with tile.TileContext(nc) as tc:
        with (
            tc.tile_pool(name="xp", bufs=1) as xp,
            tc.tile_pool(name="ap", bufs=1) as ap,
            tc.tile_pool(name="bp", bufs=1) as bp,
            tc.tile_pool(name="tp", bufs=2) as tp,
            tc.tile_pool(name="pt", bufs=2, space="PSUM") as pt,
            tc.tile_pool(name="py", bufs=4, space="PSUM") as py,
            tc.tile_pool(name="yp", bufs=2) as yp,
        ):
            loop = tc.For_i(0, iters, 1) if iters > 1 else contextlib.nullcontext()
            with loop:
                # x: [2048, 512] -> sbuf [128, 16, 512]; free block k = k-chunk
                x_sb = xp.tile([128, KCH, TPC], mdt)
                nc.sync.dma_start(
                    out=x_sb[:], in_=xT[:, :].rearrange("(k p) n -> p k n", p=128)
                )
                # a: [2, 2048, 16] -> sbuf [128, 2, 16, 16]
                a_sb = ap.tile([128, N_MOD, KCH, MAX_RANK], mdt)
                nc.sync.dma_start(
                    out=a_sb[:], in_=a[:, :, :].rearrange("m (k p) r -> p m k r", p=128)
                )
                if packed:
                    # b replicated at partition offsets {0,32,64,96} so 4
                    # row-tile matmuls (one per 128-token tile) run
                    # concurrently in disjoint 32-row strips of the PE array.
                    # One HBM load + 3 SBUF->SBUF replicas.
                    b_sb = []
                    for m in range(N_MOD):
                        bt = bp.tile([128, OUT], mdt, tag=f"b{m}")
                        nc.sync.dma_start(out=bt[0:MAX_RANK, :], in_=b[m])
                        for j in range(1, MT):
                            nc.sync.dma_start(
                                out=bt[32 * j : 32 * j + MAX_RANK, :],
                                in_=bt[0:MAX_RANK, :],
                            )
                        b_sb.append(bt)
                else:
                    b_sb = []
                    for m in range(N_MOD):
                        bt = bp.tile([MAX_RANK, OUT], mdt, tag=f"b{m}")
                        nc.sync.dma_start(out=bt[:], in_=b[m])
                        b_sb.append(bt)

                for m in range(N_MOD):
                    if packed:
                        # GEMM1: single t.T [16, 512], then replicate to
                        # partition offsets {0,32,64,96} via SBUF->SBUF DMA
                        # for the row-packed GEMM2.
                        t_ps = pt.tile([MAX_RANK, TPC], f32)
                        for k in range(KCH):
                            nc.tensor.matmul(
                                out=t_ps[:],
                                lhsT=a_sb[:, m, k, :],
                                rhs=x_sb[:, k, :],
                                start=(k == 0),
                                stop=(k == KCH - 1),
                            )
                        t_sb = tp.tile([128, TPC], mdt)
                        nc.vector.tensor_copy(t_sb[0:MAX_RANK, :], t_ps[:])
                        for j in range(1, MT):
                            nc.sync.dma_start(
                                out=t_sb[32 * j : 32 * j + MAX_RANK, :],
                                in_=t_sb[0:MAX_RANK, :],
                            )

                        # GEMM2: for each output col tile, 4 row-packed
                        # matmuls (one per token tile) run concurrently.
                        # Output staged per token tile in [128, 2048] blocks.
                        NTB = 4  # col tiles per stage block
                        for nt_blk in range(NT // NTB):
                            y_stage = [
                                yp.tile([128, NTB * 512], f32, tag=f"ystage{mt}",
                                        name=f"ystage{mt}")
                                for mt in range(MT)
                            ]
                            for nti in range(NTB):
                                nt = nt_blk * NTB + nti
                                for mt in range(MT):
                                    y_ps = py.tile([128, 512], f32)
                                    nc.tensor.matmul(
                                        out=y_ps[:],
                                        lhsT=t_sb[
                                            32 * mt : 32 * mt + MAX_RANK,
                                            mt * 128 : (mt + 1) * 128,
                                        ],
                                        rhs=b_sb[m][
                                            32 * mt : 32 * mt + MAX_RANK,
                                            nt * 512 : (nt + 1) * 512,
                                        ],
                                        start=True,
                                        stop=True,
                                        tile_position=(32 * mt, 0),
                                    )
                                    dst = y_stage[mt][:, nti * 512 : (nti + 1) * 512]
                                    if copy_mix and (nt * MT + mt) % copy_mix == copy_mix - 1:
                                        nc.scalar.copy(dst, y_ps[:])
                                    else:
                                        nc.vector.tensor_copy(dst, y_ps[:])
                            for mt in range(MT):
                                nc.sync.dma_start(
                                    out=y[
                                        mt * 128 : (mt + 1) * 128,
                                        m * OUT + nt_blk * NTB * 512 :
                                        m * OUT + (nt_blk + 1) * NTB * 512,
                                    ],
                                    in_=y_stage[mt][:],
                                )
                        continue

                    # GEMM1: t.T [16, 512] accumulated over 16 k-chunks
                    t_ps = pt.tile([MAX_RANK, TPC], f32)
                    for k in range(KCH):
                        nc.tensor.matmul(
                            out=t_ps[:],
                            lhsT=a_sb[:, m, k, :],
                            rhs=x_sb[:, k, :],
                            start=(k == 0),
                            stop=(k == KCH - 1),
                        )
                    t_sb = tp.tile([MAX_RANK, TPC], mdt)
                    nc.vector.tensor_copy(t_sb[:], t_ps[:])

                    # GEMM2: per 128-token tile, fill a [128, 8192] stage, DMA
                    for mt in range(MT):
                        y_stage = yp.tile([128, OUT], f32, tag="ystage")
                        for nt in range(NT):
                            y_ps = py.tile([128, 512], f32)
                            nc.tensor.matmul(
                                out=y_ps[:],
                                lhsT=t_sb[:, mt * 128 : (mt + 1) * 128],
                                rhs=b_sb[m][:, nt * 512 : (nt + 1) * 512],
                                start=True,
                                stop=True,
                            )
                            dst = y_stage[:, nt * 512 : (nt + 1) * 512]
                            if copy_mix and nt % copy_mix == copy_mix - 1:
                                nc.scalar.copy(dst, y_ps[:])
                            else:
                                nc.vector.tensor_copy(dst, y_ps[:])
                        nc.sync.dma_start(
                            out=y[mt * 128 : (mt + 1) * 128, m * OUT : (m + 1) * OUT],
                            in_=y_stage[:],
                        )
    nc.compile()
    return nc


def _build_bass_b3(iters=1, copy_mix=3):
    """bf16 hi/lo decomposition: every GEMM = hi@hi + hi@lo + lo@hi in bf16,
    accumulated in fp32 PSUM (error ~2^-16 relative). GEMM2 is row-packed
    4x via tile_position. Inputs arrive pre-split (host: hi = bf16(v),
    lo = bf16(v - f32(hi)))."""
    import contextlib

    import concourse.mybir as mybir
    import concourse.tile as tile
    from concourse import bacc

    f32 = mybir.dt.float32
    bf16 = mybir.dt.bfloat16
    nc = bacc.Bacc()

    xh = nc.dram_tensor("xh", [HIDDEN, TPC], bf16, kind="ExternalInput")
    xl = nc.dram_tensor("xl", [HIDDEN, TPC], bf16, kind="ExternalInput")
    ah = nc.dram_tensor("ah", [N_MOD, HIDDEN, MAX_RANK], bf16, kind="ExternalInput")
    al = nc.dram_tensor("al", [N_MOD, HIDDEN, MAX_RANK], bf16, kind="ExternalInput")
    bh = nc.dram_tensor("bh", [N_MOD, MAX_RANK, OUT], bf16, kind="ExternalInput")
    bl = nc.dram_tensor("bl", [N_MOD, MAX_RANK, OUT], bf16, kind="ExternalInput")
    y = nc.dram_tensor("y", [TPC, N_MOD * OUT], f32, kind="ExternalOutput")

    with tile.TileContext(nc) as tc:
        with (
            tc.tile_pool(name="xp", bufs=1) as xp,
            tc.tile_pool(name="ap", bufs=1) as ap,
            tc.tile_pool(name="bp", bufs=1) as bp,
            tc.tile_pool(name="tp", bufs=2) as tp,
            tc.tile_pool(name="pt", bufs=2, space="PSUM") as pt,
            tc.tile_pool(name="py", bufs=6, space="PSUM") as py,
            tc.tile_pool(name="yp", bufs=2) as yp,
        ):
            loop = tc.For_i(0, iters, 1) if iters > 1 else contextlib.nullcontext()
            with loop:
                x_sb = xp.tile([128, 2, KCH, TPC], bf16)
                for i, xsrc in enumerate((xh, xl)):
                    nc.sync.dma_start(
                        out=x_sb[:, i],
                        in_=xsrc[:, :].rearrange("(k p) n -> p k n", p=128),
                    )
                a_sb = ap.tile([128, 2, N_MOD, KCH, MAX_RANK], bf16)
                for i, asrc in enumerate((ah, al)):
                    nc.sync.dma_start(
                        out=a_sb[:, i],
                        in_=asrc[:, :, :].rearrange("m (k p) r -> p m k r", p=128),
                    )
                # b hi/lo replicated at partition offsets {0,32,64,96}
                b_sb = []  # [m][hl] -> tile [128, OUT]
                for m in range(N_MOD):
                    pair = []
                    for i, bsrc in enumerate((bh, bl)):
                        bt = bp.tile([128, OUT], bf16, tag=f"b{m}{i}",
                                     name=f"b{m}{i}")
                        nc.sync.dma_start(out=bt[0:MAX_RANK, :], in_=bsrc[m])
                        for j in range(1, MT):
                            nc.sync.dma_start(
                                out=bt[32 * j : 32 * j + MAX_RANK, :],
                                in_=bt[0:MAX_RANK, :],
                            )
                        pair.append(bt)
                    b_sb.append(pair)

                for m in range(N_MOD):
                    # GEMM1: t.T = sum_k (ah@xh + ah@xl + al@xh), fp32 PSUM
                    t_ps = pt.tile([MAX_RANK, TPC], f32)
                    n_mm = KCH * 3
                    i_mm = 0
                    for k in range(KCH):
                        for ia, ix in ((0, 0), (0, 1), (1, 0)):
                            nc.tensor.matmul(
                                out=t_ps[:],
                                lhsT=a_sb[:, ia, m, k, :],
                                rhs=x_sb[:, ix, k, :],
                                start=(i_mm == 0),
                                stop=(i_mm == n_mm - 1),
                            )
                            i_mm += 1
                    # split t into bf16 hi/lo and replicate to 4 offsets
                    t_hi = tp.tile([128, TPC], bf16, tag="t_hi", name="t_hi")
                    t_lo = tp.tile([128, TPC], bf16, tag="t_lo", name="t_lo")
                    nc.vector.tensor_copy(t_hi[0:MAX_RANK, :], t_ps[:])
                    nc.vector.tensor_sub(
                        t_lo[0:MAX_RANK, :], t_ps[:], t_hi[0:MAX_RANK, :]
                    )
                    for tt in (t_hi, t_lo):
                        for j in range(1, MT):
                            nc.sync.dma_start(
                                out=tt[32 * j : 32 * j + MAX_RANK, :],
                                in_=tt[0:MAX_RANK, :],
                            )

                    # GEMM2: 3-term accumulation, row-packed 4x over mt
                    NTB = 4
                    for nt_blk in range(NT // NTB):
                        y_stage = [
                            yp.tile([128, NTB * 512], f32, tag=f"ystage{mt}",
                                    name=f"ystage{mt}")
                            for mt in range(MT)
                        ]
                        for nti in range(NTB):
                            nt = nt_blk * NTB + nti
                            for mt in range(MT):
                                y_ps = py.tile([128, 512], f32)
                                for gi, (tt, bb) in enumerate(
                                    ((t_hi, b_sb[m][0]),
                                     (t_hi, b_sb[m][1]),
                                     (t_lo, b_sb[m][0]))
                                ):
                                    nc.tensor.matmul(
                                        out=y_ps[:],
                                        lhsT=tt[
                                            32 * mt : 32 * mt + MAX_RANK,
                                            mt * 128 : (mt + 1) * 128,
                                        ],
                                        rhs=bb[
                                            32 * mt : 32 * mt + MAX_RANK,
                                            nt * 512 : (nt + 1) * 512,
                                        ],
                                        start=(gi == 0),
                                        stop=(gi == 2),
                                        tile_position=(32 * mt, 0),
                                    )
                                dst = y_stage[mt][:, nti * 512 : (nti + 1) * 512]
                                if copy_mix and (nt * MT + mt) % copy_mix == copy_mix - 1:
                                    nc.scalar.copy(dst, y_ps[:])
                                else:
                                    nc.vector.tensor_copy(dst, y_ps[:])
                        for mt in range(MT):
                            nc.sync.dma_start(
                                out=y[
                                    mt * 128 : (mt + 1) * 128,
                                    m * OUT + nt_blk * NTB * 512 :
                                    m * OUT + (nt_blk + 1) * NTB * 512,
                                ],
                                in_=y_stage[mt][:],
                            )
    nc.compile()
    return nc


def _build_bass_b3s(iters=1, copy_mix=3):
    """Like b3 but GEMM2 stacks the three bf16 terms along the contraction
    dim: K=48 = [t_hi|t_hi|t_lo] against [B_hi|B_lo|B_hi], one matmul per
    output tile, duplicated in two 64-partition K-blocks for 2-way
    tile_position packing."""
    import contextlib

    import concourse.mybir as mybir
    import concourse.tile as tile
    from concourse import bacc

    f32 = mybir.dt.float32
    bf16 = mybir.dt.bfloat16
    R = MAX_RANK
    nc = bacc.Bacc()

    xh = nc.dram_tensor("xh", [HIDDEN, TPC], bf16, kind="ExternalInput")
    xl = nc.dram_tensor("xl", [HIDDEN, TPC], bf16, kind="ExternalInput")
    ah = nc.dram_tensor("ah", [N_MOD, HIDDEN, MAX_RANK], bf16, kind="ExternalInput")
    al = nc.dram_tensor("al", [N_MOD, HIDDEN, MAX_RANK], bf16, kind="ExternalInput")
    bh = nc.dram_tensor("bh", [N_MOD, MAX_RANK, OUT], bf16, kind="ExternalInput")
    bl = nc.dram_tensor("bl", [N_MOD, MAX_RANK, OUT], bf16, kind="ExternalInput")
    y = nc.dram_tensor("y", [TPC, N_MOD * OUT], f32, kind="ExternalOutput")

    with tile.TileContext(nc) as tc:
        with (
            tc.tile_pool(name="xp", bufs=1) as xp,
            tc.tile_pool(name="ap", bufs=1) as ap,
            tc.tile_pool(name="bp", bufs=1) as bp,
            tc.tile_pool(name="tp", bufs=2) as tp,
            tc.tile_pool(name="pt", bufs=2, space="PSUM") as pt,
            tc.tile_pool(name="py", bufs=6, space="PSUM") as py,
            tc.tile_pool(name="yp", bufs=2) as yp,
        ):
            loop = tc.For_i(0, iters, 1) if iters > 1 else contextlib.nullcontext()
            with loop:
                x_sb = xp.tile([128, 2, KCH, TPC], bf16)
                for i, xsrc in enumerate((xh, xl)):
                    nc.sync.dma_start(
                        out=x_sb[:, i],
                        in_=xsrc[:, :].rearrange("(k p) n -> p k n", p=128),
                    )
                a_sb = ap.tile([128, 2, N_MOD, KCH, MAX_RANK], bf16)
                for i, asrc in enumerate((ah, al)):
                    nc.sync.dma_start(
                        out=a_sb[:, i],
                        in_=asrc[:, :, :].rearrange("m (k p) r -> p m k r", p=128),
                    )
                # b stacked [B_hi|B_lo|B_hi] at partitions 0-47, replicated
                # to 64-111 for 2-way row packing.
                b_sb = []
                for m in range(N_MOD):
                    bt = bp.tile([128, OUT], bf16, tag=f"bs{m}", name=f"bs{m}")
                    nc.sync.dma_start(out=bt[0:R, :], in_=bh[m])
                    nc.sync.dma_start(out=bt[R : 2 * R, :], in_=bl[m])
                    nc.sync.dma_start(out=bt[2 * R : 3 * R, :], in_=bh[m])
                    nc.sync.dma_start(out=bt[64 : 64 + 3 * R, :], in_=bt[0 : 3 * R, :])
                    b_sb.append(bt)

                # GEMM1: the two modules' 48-MM accumulation chains go to
                # col groups 0 and 1 (psum partitions 0-15 / 32-47) so the
                # PE streams them concurrently.
                TERMS = ((0, 0), (0, 1), (1, 0))  # (a hi/lo, x hi/lo)
                g1_ps = pt.tile([64, TPC], f32, tag="g1ps", name="g1ps")
                for m in range(N_MOD):
                    strip = g1_ps[32 * m : 32 * m + MAX_RANK, :]
                    n_mm = KCH * len(TERMS)
                    i_mm = 0
                    for k in range(KCH):
                        for ia, ix in TERMS:
                            nc.tensor.matmul(
                                out=strip,
                                lhsT=a_sb[:, ia, m, k, :],
                                rhs=x_sb[:, ix, k, :],
                                start=(i_mm == 0),
                                stop=(i_mm == n_mm - 1),
                                tile_position=(0, 32 * m),
                            )
                            i_mm += 1

                for m in range(N_MOD):
                    strip = g1_ps[32 * m : 32 * m + MAX_RANK, :]
                    # split into hi/lo and assemble K-stacked t_cat
                    t_hi = tp.tile([64, TPC], bf16, tag="t_hi", name="t_hi")
                    t_lo = tp.tile([64, TPC], bf16, tag="t_lo", name="t_lo")
                    t_hi_s = t_hi[32 * m : 32 * m + MAX_RANK, :]
                    t_lo_s = t_lo[32 * m : 32 * m + MAX_RANK, :]
                    nc.vector.tensor_copy(t_hi_s, strip)
                    nc.vector.tensor_sub(t_lo_s, strip, t_hi_s)
                    t_cat = tp.tile([128, TPC], bf16, tag="t_cat", name="t_cat")
                    nc.sync.dma_start(out=t_cat[0:R, :], in_=t_hi_s)
                    nc.sync.dma_start(out=t_cat[R : 2 * R, :], in_=t_hi_s)
                    nc.sync.dma_start(out=t_cat[2 * R : 3 * R, :], in_=t_lo_s)
                    nc.sync.dma_start(
                        out=t_cat[64 : 64 + 3 * R, :], in_=t_cat[0 : 3 * R, :]
                    )

                    # GEMM2: one K=48 matmul per (mt, nt), 2-way packed
                    NTB = 4
                    for nt_blk in range(NT // NTB):
                        y_stage = [
                            yp.tile([128, NTB * 512], f32, tag=f"ystage{mt}",
                                    name=f"ystage{mt}")
                            for mt in range(MT)
                        ]
                        for nti in range(NTB):
                            nt = nt_blk * NTB + nti
                            for mt in range(MT):
                                blk = 64 * (mt % 2)
                                y_ps = py.tile([128, 512], f32)
                                nc.tensor.matmul(
                                    out=y_ps[:],
                                    lhsT=t_cat[
                                        blk : blk + 3 * R,
                                        mt * 128 : (mt + 1) * 128,
                                    ],
                                    rhs=b_sb[m][
                                        blk : blk + 3 * R,
                                        nt * 512 : (nt + 1) * 512,
                                    ],
                                    start=True,
                                    stop=True,
                                    tile_position=(blk, 0),
                                )
                                dst = y_stage[mt][:, nti * 512 : (nti + 1) * 512]
                                if copy_mix and (nt * MT + mt) % copy_mix == copy_mix - 1:
                                    nc.scalar.copy(dst, y_ps[:])
                                else:
                                    nc.vector.tensor_copy(dst, y_ps[:])
                        for mt in range(MT):
                            nc.sync.dma_start(
                                out=y[
                                    mt * 128 : (mt + 1) * 128,
                                    m * OUT + nt_blk * NTB * 512 :
                                    m * OUT + (nt_blk + 1) * NTB * 512,
                                ],
                                in_=y_stage[mt][:],
                            )
    nc.compile()
    return nc


def split_bf16(v):
    import ml_dtypes

    hi = v.astype(ml_dtypes.bfloat16)
    lo = (v - hi.astype(np.float32)).astype(ml_dtypes.bfloat16)
    return hi, lo


def get_bass(iters=1, mm_dtype="float32", copy_mix=3, packed=False):
    key = ("nc", iters, mm_dtype, copy_mix, packed)
    if key not in _CACHE:
        if mm_dtype == "b3":
            _CACHE[key] = _build_bass_b3(iters, copy_mix)
        elif mm_dtype == "b3s":
            _CACHE[key] = _build_bass_b3s(iters, copy_mix)
        else:
            _CACHE[key] = _build_bass(iters, mm_dtype, copy_mix, packed)
    return _CACHE[key]


def _host_prep(x, A, B, sorted_ids, slot_ranks, slot_offsets, b3=True):
    """Shard: per-core gathered/transposed x + per-slot rank-masked weights.

    Returns in_maps (list of 8 dicts) or None if the slot layout doesn't
    align with 512-row blocks (then the caller falls back to numpy).
    With b3=True, tensors are bf16 hi/lo splits for the b3s kernel."""
    bs = x.shape[0]
    row_slot = (
        np.searchsorted(slot_offsets, np.arange(bs), side="right").astype(np.int64) - 1
    )
    rmask = (
        np.arange(MAX_RANK)[None, :] < slot_ranks[:, None].astype(np.int64)
    ).astype(x.dtype)
    Am = A * rmask[None, :, None, :]  # [m, s, h, r]

    in_maps = []
    for c in range(N_CORES):
        rows = slice(c * TPC, (c + 1) * TPC)
        slots = row_slot[rows]
        s0 = int(slots[0])
        if not (slots == s0).all():
            return None
        ids = sorted_ids[rows]
        xT_c = np.ascontiguousarray(x[ids].T)
        a_c = np.ascontiguousarray(Am[:, s0])
        b_c = np.ascontiguousarray(B[:, s0])
        if b3:
            xh, xl = split_bf16(xT_c)
            ah, al = split_bf16(a_c)
            bh, bl = split_bf16(b_c)
            in_maps.append(
                {"xh": xh, "xl": xl, "ah": ah, "al": al, "bh": bh, "bl": bl}
            )
        else:
            in_maps.append({"xT": xT_c, "a": a_c, "b": b_c})
    return in_maps


def _numpy_fallback(x, A, B, sorted_ids, slot_ranks, slot_offsets):
    bs = x.shape[0]
    xr = x[sorted_ids]
    row_slot = (
        np.searchsorted(slot_offsets, np.arange(bs), side="right").astype(np.int64) - 1
    )
    rmask = (
        np.arange(MAX_RANK)[None, :] < slot_ranks[:, None].astype(np.int64)
    ).astype(x.dtype)
    Am = A * rmask[None, :, None, :]
    out = np.zeros((bs, N_MOD * OUT), dtype=x.dtype)
    for s in range(N_SLOTS):
        sel = row_slot == s
        if not sel.any():
            continue
        xs = xr[sel]
        for m in range(N_MOD):
            out[sel, m * OUT : (m + 1) * OUT] = (xs @ Am[m, s]) @ B[m, s]
    return out


def kernel(
    x,
    A,
    B,
    sorted_ids,
    slot_counts=None,
    slot_ranks=None,
    slot_offsets=None,
    **_unused,
):
    x = np.asarray(x, dtype=np.float32)
    A = np.asarray(A, dtype=np.float32)
    B = np.asarray(B, dtype=np.float32)
    sorted_ids = np.asarray(sorted_ids)
    slot_ranks = np.asarray(slot_ranks)
    slot_offsets = np.asarray(slot_offsets)

    in_maps = _host_prep(x, A, B, sorted_ids, slot_ranks, slot_offsets, b3=True)
    if in_maps is None:
        return _numpy_fallback(x, A, B, sorted_ids, slot_ranks, slot_offsets)

    from concourse import bass_utils

    nc = get_bass(mm_dtype="b3s", copy_mix=2)
    res = bass_utils.run_bass_kernel_spmd(nc, in_maps, core_ids=list(range(N_CORES)))
    return np.concatenate([r["y"] for r in res.results], axis=0)


if __name__ == "__main__":
    rng = np.random.default_rng(0)
    x = rng.standard_normal((BS, HIDDEN), dtype=np.float32)
    A = rng.standard_normal((N_MOD, N_SLOTS, HIDDEN, MAX_RANK), dtype=np.float32) * 0.02
    B = rng.standard_normal((N_MOD, N_SLOTS, MAX_RANK, OUT), dtype=np.float32) * 0.02
    ids = rng.permutation(BS).astype(np.int32)
    counts = np.full(N_SLOTS, BS // N_SLOTS, np.int32)
    offs = np.concatenate([[0], np.cumsum(counts)[:-1]]).astype(np.int32)
    ranks = np.array([16, 8, 16, 4, 16, 8, 12, 16], np.int32)
    out = kernel(
        x=x, A=A, B=B, sorted_ids=ids, slot_counts=counts,
        slot_ranks=ranks, slot_offsets=offs,
    )
    ref = _numpy_fallback(x, A, B, ids, ranks, offs)
    err = np.abs(out - ref).max() / (np.abs(ref).max() + 1e-30)
    print("max rel err vs numpy:", err)
